# revision 2
# baseline (speedup 1.0000x reference)
"""EnhancedMultiHeadAttention on 8 Trainium2 NeuronCores — v2.

Sharding: 8 cores = 2 batches x 4 head-groups (4 heads / 256 columns each).

Design notes:
- bf16 on-chip everywhere (host-cast bf16 inputs); fp32 PSUM accumulation.
- Stream order xk, xq, xv: K/Q projections chase their streams so the
  exp pipeline (the throughput bottleneck: 128 x ~1.15us ScalarE
  activations) starts as soon as xq lands.  The V projection is
  interleaved into the first ~20 attention steps while xv streams, and
  AV matmuls trail the exp stream by SKEW=20 steps (pe tiles buffer in
  a deep SBUF ring).
- All x tiles go through one 16-slot SBUF ring; xv reuses xk's slots.
- V projection computed directly in [token, channel] layout (lhsT = x
  tile) in 4-bank PSUM quarters; no PE transposes.
- Projection bias epilogues on VectorE so ScalarE does exp only.
- Softmax normalization: copy av PSUM->SBUF (frees the PSUM ring fast),
  reciprocal of the denominator row, then a K=1 PE matmul broadcasts
  the reciprocal row across 64 partitions — no DRAM bounce.  The PE
  broadcast is scheduled 16 steps later so it never waits on the
  reciprocal.
- exp(temporal_bias) folded into V' and the denominator column.
- A PE warm-up burst trips the HAM clock gate to 8/8 before the
  projections start.
"""

import sys

for _p in ("/opt/trn_rl_repo", "/root/.axon_site/_ro/trn_rl_repo"):
    if _p not in sys.path:
        sys.path.append(_p)

import numpy as np
import ml_dtypes

import concourse.bass as bass
import concourse.mybir as mybir
import concourse.tile as tile
from concourse import bacc
from concourse.bass_utils import run_bass_kernel_spmd

F32 = mybir.dt.float32
BF16 = mybir.dt.bfloat16

B, S, D = 2, 2048, 1024
H, DEPTH = 16, 64
NCORES = 8
GROUPS = 4                  # head-groups per batch
HC = H // GROUPS            # heads per core = 4
C = HC * DEPTH              # columns per core = 256
NPAIR = HC // 2             # head pairs per core = 2
DT = D // 128               # 8 d-tiles
TT = S // 128               # 16 token tiles
QB = S // 512               # 4 q blocks
KT = S // 128               # 16 k tiles
SCALE = 0.125               # 1/sqrt(DEPTH)

SKEW = 16     # AV trails exp by this many flat steps (pe ring buffers)
VEND = 16     # flat steps reserved for interleaved V projection
NORM2D = 12   # steps from a unit's last AV to its bc broadcast
OPD = 20      # steps from a qb's last AV to its first out-proj chain
OPSPACE = 2


def build_nc():
    nc = bacc.Bacc(None, target_bir_lowering=False)

    xq = nc.dram_tensor("xq", [D, S], BF16, kind="ExternalInput")
    xk = nc.dram_tensor("xk", [D, S], BF16, kind="ExternalInput")
    xv = nc.dram_tensor("xv", [D, S], BF16, kind="ExternalInput")
    wq = nc.dram_tensor("wq", [D, C], BF16, kind="ExternalInput")
    wk = nc.dram_tensor("wk", [D, C], BF16, kind="ExternalInput")
    wv = nc.dram_tensor("wv", [D, C], BF16, kind="ExternalInput")
    wo = nc.dram_tensor("wo", [C, D], BF16, kind="ExternalInput")
    bq = nc.dram_tensor("bq", [C], F32, kind="ExternalInput")
    bk = nc.dram_tensor("bk", [C], F32, kind="ExternalInput")
    bv = nc.dram_tensor("bv", [C], F32, kind="ExternalInput")
    expb = nc.dram_tensor("expb", [S], F32, kind="ExternalInput")
    out = nc.dram_tensor("out", [S, D], F32, kind="ExternalOutput")

    with tile.TileContext(nc) as tc, nc.allow_low_precision(
        reason="bf16 storage with fp32 PSUM accumulation throughout"
    ):
        with (
            tc.tile_pool(name="wpool", bufs=1) as wp,
            tc.tile_pool(name="qk", bufs=1) as qkp,
            tc.tile_pool(name="vsb", bufs=1) as vp_,
            tc.tile_pool(name="ctxp", bufs=1) as cxp,
            tc.tile_pool(name="xs", bufs=16) as xsp,
            tc.tile_pool(name="vep", bufs=2) as vep,
            tc.tile_pool(name="pex", bufs=SKEW + 4) as pex,
            tc.tile_pool(name="nrm", bufs=2) as nrm,
            tc.tile_pool(name="osb", bufs=3) as osb,
        ):
            # ---- DMA issue order on the sync FIFO sets stream pacing ----
            wk_sb = wp.tile([128, DT, C], BF16)
            wq_sb = wp.tile([128, DT, C], BF16)
            nc.sync.dma_start(wk_sb[:], wk.rearrange("(dt p) c -> p dt c", p=128))
            nc.sync.dma_start(wq_sb[:], wq.rearrange("(dt p) c -> p dt c", p=128))
            xkr = xk.rearrange("(dt p) t -> dt p t", p=128)
            xqr = xq.rearrange("(dt p) t -> dt p t", p=128)
            xvr = xv.rearrange("(dt p) t -> dt p t", p=128)
            xk_t, xq_t, xv_t = [], [], []
            for dt in range(DT):
                xt = xsp.tile([128, S], BF16, tag="xt", name="xkt")
                nc.sync.dma_start(xt[:], xkr[dt])
                xk_t.append(xt)
            for dt in range(DT):
                xt = xsp.tile([128, S], BF16, tag="xt", name="xqt")
                nc.sync.dma_start(xt[:], xqr[dt])
                xq_t.append(xt)
            bq_sb = wp.tile([128, 2], F32)
            bk_sb = wp.tile([128, 2], F32)
            nc.sync.dma_start(bq_sb[:], bq.rearrange("(ct p) -> p ct", p=128))
            nc.sync.dma_start(bk_sb[:], bk.rearrange("(ct p) -> p ct", p=128))
            bvr_sb = wp.tile([128, C], F32)
            bv_ap = bv[:]
            bv_bc = bass.AP(tensor=bv_ap.tensor, offset=bv_ap.offset,
                            ap=[[0, 128]] + [list(a) for a in bv_ap.ap])
            nc.sync.dma_start(bvr_sb[:], bv_bc)
            expb_sb = wp.tile([128, TT], F32)
            nc.sync.dma_start(expb_sb[:], expb.rearrange("(tt p) -> p tt", p=128))
            wv_sb = wp.tile([128, DT, C], BF16)
            nc.sync.dma_start(wv_sb[:], wv.rearrange("(dt p) c -> p dt c", p=128))
            for dt in range(DT):
                xt = xsp.tile([128, S], BF16, tag="xt", name="xvt")
                nc.sync.dma_start(xt[:], xvr[dt])
                xv_t.append(xt)
            wo_sb = wp.tile([128, 2, D], BF16)
            nc.sync.dma_start(wo_sb[:], wo.rearrange("(ct p) n -> p ct n", p=128))

            ones65 = wp.tile([65, 64], BF16)
            nc.gpsimd.memset(ones65[:], 1.0)

            # ---- PE warm-up: trips HAM to K=8/8 before the projections ----
            with tc.tile_pool(name="warm", bufs=1, space="PSUM") as wmp:
                wps_t = wmp.tile([64, 64], F32)
                for _ in range(44):
                    nc.tensor.matmul(
                        wps_t[:], ones65[64:65, 0:64], ones65[64:65, 0:64],
                        start=True, stop=True,
                    )

            # ---- persistent activations ----
            qT = [qkp.tile([128, S], BF16, tag=f"qT{i}", name=f"qT{i}") for i in range(NPAIR)]
            kT = [qkp.tile([128, S], BF16, tag=f"kT{i}", name=f"kT{i}") for i in range(NPAIR)]
            vs = [vp_.tile([128, HC, 65], BF16, tag=f"vs{t}", name=f"vs{t}") for t in range(TT)]
            ctx = [cxp.tile([128, S], BF16, tag=f"ctx{i}", name=f"ctx{i}") for i in range(NPAIR)]

            # ================= K / Q projections (chase their streams) ======
            with tc.tile_pool(name="pp", bufs=8, space="PSUM") as pp:
                for xts, w_sb, b_sb, dst in (
                    (xk_t, wk_sb, bk_sb, kT),
                    (xq_t, wq_sb, bq_sb, qT),
                ):
                    ps = {}
                    for dt in range(DT):
                        xt = xts[dt]
                        for ct in range(2):
                            for tb in range(QB):
                                if dt == 0:
                                    ps[ct, tb] = pp.tile([128, 512], F32, tag="pp", name=f"ps{ct}_{tb}")
                                nc.tensor.matmul(
                                    ps[ct, tb][:],
                                    w_sb[:, dt, ct * 128:(ct + 1) * 128],
                                    xt[:, tb * 512:(tb + 1) * 512],
                                    start=(dt == 0),
                                    stop=(dt == DT - 1),
                                )
                                if dt == DT - 1:
                                    nc.vector.tensor_scalar_add(
                                        dst[ct][:, tb * 512:(tb + 1) * 512],
                                        ps[ct, tb][:],
                                        b_sb[:, ct:ct + 1],
                                    )

            # ========== attention w/ interleaved V-proj + output proj =======
            oout = out.rearrange("(qt p) n -> qt p n", p=128)
            flat = [(qb, pr, kt) for qb in range(QB) for pr in range(NPAIR)
                    for kt in range(KT)]
            NF = len(flat)
            pending = {}
            avs = {}
            norm_state = {}

            # ---- V-projection work schedule over the first VEND steps ----
            # quarter 0 (toks 0-3) chases the xv stream dt by dt;
            # quarters 1-3 (toks 4-15) run from resident tiles, one tok/step.
            vsched = {}
            for dt in range(DT):
                vsched.setdefault(min(dt, VEND - 1), []).append(("q0dt", dt))
            for t in range(4, TT):
                vsched.setdefault(min(8 + (t - 4) * 8 // 12, VEND - 1),
                                  []).append(("tok", t))

            norm2_sched = {}
            op_sched = {}
            for u in range(QB * NPAIR):
                pos = (u + 1) * KT - 1 + SKEW + NORM2D
                norm2_sched.setdefault(pos, []).append((u // NPAIR, u % NPAIR))
            for qb in range(QB):
                last_av = (qb + 1) * NPAIR * KT - 1 + SKEW
                for j in range(8):
                    op_sched.setdefault(last_av + OPD + j * OPSPACE, []).append((qb, j))

            with tc.tile_pool(name="sps", bufs=2, space="PSUM") as sps:

                def emit_qk(qb, pr, kt):
                    qsl = slice(qb * 512, (qb + 1) * 512)
                    ksl = slice(kt * 128, (kt + 1) * 128)
                    st = sps.tile([128, 1024], F32, tag="s", name="st")
                    for hh in range(2):
                        psl = slice(hh * 64, (hh + 1) * 64)
                        nc.tensor.matmul(
                            st[:, hh * 512:(hh + 1) * 512],
                            kT[pr][psl, ksl],
                            qT[pr][psl, qsl],
                        )
                    pe = pex.tile([128, 1024], BF16, tag="pe", name="pe")
                    nc.scalar.activation(
                        pe[:], st[:], mybir.ActivationFunctionType.Exp, scale=SCALE
                    )
                    pending[qb, pr, kt] = pe

                def emit_norm(qb, pr):
                    av = avs.pop((qb, pr))
                    avc = []
                    recs = []
                    for hh in range(2):
                        a = nrm.tile([65, 512], F32, tag="avc", bufs=4, name="avc")
                        nc.vector.tensor_copy(a[:], av[hh][:])
                        avc.append(a)
                    for hh in range(2):
                        rec = nrm.tile([65, 512], BF16, tag="rec", bufs=4, name="rec")
                        nc.vector.reciprocal(rec[64:65, :], avc[hh][64:65, :])
                        recs.append(rec)
                    norm_state[qb, pr] = (avc, recs)

                def emit_vtok_epi(vpt, tt):
                    tmpv = vep.tile([128, C], F32, tag="tmpv", name="tmpv")
                    nc.vector.tensor_add(tmpv[:], vpt[:], bvr_sb[:])
                    nc.vector.tensor_scalar_mul(
                        vs[tt][:, :, 0:64],
                        tmpv[:].rearrange("p (h d) -> p h d", h=HC),
                        expb_sb[:, tt:tt + 1],
                    )
                    for h in range(HC):
                        nc.gpsimd.tensor_copy(
                            vs[tt][:, h, 64:65], expb_sb[:, tt:tt + 1]
                        )

                # ---- phase A: QK/exp + interleaved V projection ----
                with tc.tile_pool(name="vps", bufs=4, space="PSUM") as vpp:
                    vq0 = [vpp.tile([128, C], F32, tag="vps", name=f"vq{j}")
                           for j in range(4)]
                    for i in range(VEND):
                        emit_qk(*flat[i])
                        for kind, a in vsched.get(i, ()):
                            if kind == "q0dt":
                                dt = a
                                for tok in range(4):
                                    nc.tensor.matmul(
                                        vq0[tok][:],
                                        xv_t[dt][:, tok * 128:(tok + 1) * 128],
                                        wv_sb[:, dt, :],
                                        start=(dt == 0),
                                        stop=(dt == DT - 1),
                                    )
                                if dt == DT - 1:
                                    for tok in range(4):
                                        emit_vtok_epi(vq0[tok], tok)
                            else:
                                t = a
                                vpt = vpp.tile([128, C], F32, tag="vps", name="vpt")
                                for dt in range(DT):
                                    nc.tensor.matmul(
                                        vpt[:],
                                        xv_t[dt][:, t * 128:(t + 1) * 128],
                                        wv_sb[:, dt, :],
                                        start=(dt == 0),
                                        stop=(dt == DT - 1),
                                    )
                                emit_vtok_epi(vpt, t)

                # ---- phase B: steady state + drain ----
                with tc.tile_pool(name="wps", bufs=4, space="PSUM") as wps:

                    def emit_av(qb, pr, kt):
                        pe = pending.pop((qb, pr, kt))
                        if kt == 0:
                            avs[qb, pr] = [
                                wps.tile([65, 512], F32, tag="w", name="av")
                                for _ in range(2)
                            ]
                        av = avs[qb, pr]
                        for hh in range(2):
                            nc.tensor.matmul(
                                av[hh][:],
                                vs[kt][:, pr * 2 + hh, :],
                                pe[:, hh * 512:(hh + 1) * 512],
                                start=(kt == 0),
                                stop=(kt == KT - 1),
                            )
                        if kt == KT - 1:
                            emit_norm(qb, pr)

                    def emit_norm2(qb, pr):
                        qsl = slice(qb * 512, (qb + 1) * 512)
                        avc, recs = norm_state.pop((qb, pr))
                        for hh in range(2):
                            bc = wps.tile([128, 512], F32, tag="w", name="bc")
                            nc.tensor.matmul(
                                bc[0:64, :], ones65[64:65, 0:64],
                                recs[hh][64:65, :],
                                start=True, stop=True,
                            )
                            if hh == 0:
                                nc.vector.tensor_mul(
                                    ctx[pr][0:64, qsl], avc[hh][0:64, :],
                                    bc[0:64, :]
                                )
                            else:
                                tmp = nrm.tile([64, 512], BF16, tag="tmp",
                                               name="tmp")
                                nc.vector.tensor_mul(tmp[:], avc[hh][0:64, :],
                                                     bc[0:64, :])
                                nc.sync.dma_start(ctx[pr][64:128, qsl], tmp[:])

                    def emit_outproj_chain(qb, j):
                        qt = qb * 4 + j // 2
                        n = j % 2
                        qts = slice(qt * 128, (qt + 1) * 128)
                        po = wps.tile([128, 512], F32, tag="w", name="po")
                        for ct in range(2):
                            nc.tensor.matmul(
                                po[:],
                                ctx[ct][:, qts],
                                wo_sb[:, ct, n * 512:(n + 1) * 512],
                                start=(ct == 0),
                                stop=(ct == 1),
                            )
                        ot = osb.tile([128, 512], F32, tag="ot", name="ot")
                        nc.vector.tensor_copy(ot[:], po[:])
                        nc.sync.dma_start(oout[qt][:, n * 512:(n + 1) * 512],
                                          ot[:])

                    emitted_ops = set()
                    for i in range(VEND, NF + SKEW):
                        if i < NF:
                            emit_qk(*flat[i])
                        for key in norm2_sched.get(i, ()):
                            emit_norm2(*key)
                        if i >= SKEW:
                            emit_av(*flat[i - SKEW])
                        for key in op_sched.get(i, ()):
                            emit_outproj_chain(*key)
                            emitted_ops.add(key)
                    for key in sorted(norm_state):
                        emit_norm2(*key)
                    for qb in range(QB):
                        for j in range(8):
                            if (qb, j) not in emitted_ops:
                                emit_outproj_chain(qb, j)

    nc.finalize()
    return nc


_NC = None


def _get_nc():
    global _NC
    if _NC is None:
        _NC = build_nc()
    return _NC


def make_in_maps(query, key, value, temporal_bias, wq, bq, wk, bk, wv, bv, wo, bo):
    f = np.float32
    bf = ml_dtypes.bfloat16
    xt = {}
    for b in range(B):
        xt["q", b] = np.ascontiguousarray(np.asarray(query[b], f).T.astype(bf))
        xt["k", b] = np.ascontiguousarray(np.asarray(key[b], f).T.astype(bf))
        xt["v", b] = np.ascontiguousarray(np.asarray(value[b], f).T.astype(bf))
    expb = np.exp(np.asarray(temporal_bias, f))
    wq = np.asarray(wq, f).astype(bf)
    wk = np.asarray(wk, f).astype(bf)
    wv = np.asarray(wv, f).astype(bf)
    wo = np.asarray(wo, f).astype(bf)
    in_maps = []
    for core in range(NCORES):
        b, g = divmod(core, GROUPS)
        cs = slice(g * C, (g + 1) * C)
        in_maps.append({
            "xq": xt["q", b],
            "xk": xt["k", b],
            "xv": xt["v", b],
            "wq": np.ascontiguousarray(wq[:, cs]),
            "wk": np.ascontiguousarray(wk[:, cs]),
            "wv": np.ascontiguousarray(wv[:, cs]),
            "wo": np.ascontiguousarray(wo[cs, :]),
            "bq": np.ascontiguousarray(np.asarray(bq, f)[cs]),
            "bk": np.ascontiguousarray(np.asarray(bk, f)[cs]),
            "bv": np.ascontiguousarray(np.asarray(bv, f)[cs]),
            "expb": np.ascontiguousarray(expb[b]),
        })
    return in_maps


def gather(results, bo):
    bo = np.asarray(bo, np.float32)
    out = np.zeros((B, S, D), np.float32)
    for core in range(NCORES):
        b = core // GROUPS
        out[b] += results[core]["out"]
    out += bo[None, None, :]
    return out


def kernel(query, key, value, temporal_bias, wq, bq, wk, bk, wv, bv, wo, bo,
           _trace=False):
    nc = _get_nc()
    in_maps = make_in_maps(query, key, value, temporal_bias,
                           wq, bq, wk, bk, wv, bv, wo, bo)
    res = run_bass_kernel_spmd(nc, in_maps, list(range(NCORES)), trace=_trace)
    out = gather(res.results, bo)
    if _trace:
        return out, res
    return out


# revision 3
# speedup vs baseline: 1.0026x; 1.0026x over previous
"""EnhancedMultiHeadAttention on 8 Trainium2 NeuronCores — v2.

Sharding: 8 cores = 2 batches x 4 head-groups (4 heads / 256 columns each).

Design notes:
- bf16 on-chip everywhere (host-cast bf16 inputs); fp32 PSUM accumulation.
- Stream order xk, xq, xv: K/Q projections chase their streams so the
  exp pipeline (the throughput bottleneck: 128 x ~1.15us ScalarE
  activations) starts as soon as xq lands.  The V projection is
  interleaved into the first ~20 attention steps while xv streams, and
  AV matmuls trail the exp stream by SKEW=20 steps (pe tiles buffer in
  a deep SBUF ring).
- All x tiles go through one 16-slot SBUF ring; xv reuses xk's slots.
- V projection computed directly in [token, channel] layout (lhsT = x
  tile) in 4-bank PSUM quarters; no PE transposes.
- Projection bias epilogues on VectorE so ScalarE does exp only.
- Softmax normalization: copy av PSUM->SBUF (frees the PSUM ring fast),
  reciprocal of the denominator row, then a K=1 PE matmul broadcasts
  the reciprocal row across 64 partitions — no DRAM bounce.  The PE
  broadcast is scheduled 16 steps later so it never waits on the
  reciprocal.
- exp(temporal_bias) folded into V' and the denominator column.
- A PE warm-up burst trips the HAM clock gate to 8/8 before the
  projections start.
"""

import sys

for _p in ("/opt/trn_rl_repo", "/root/.axon_site/_ro/trn_rl_repo"):
    if _p not in sys.path:
        sys.path.append(_p)

import numpy as np
import ml_dtypes

import concourse.bass as bass
import concourse.mybir as mybir
import concourse.tile as tile
from concourse import bacc
from concourse.bass_utils import run_bass_kernel_spmd

F32 = mybir.dt.float32
BF16 = mybir.dt.bfloat16

B, S, D = 2, 2048, 1024
H, DEPTH = 16, 64
NCORES = 8
GROUPS = 4                  # head-groups per batch
HC = H // GROUPS            # heads per core = 4
C = HC * DEPTH              # columns per core = 256
NPAIR = HC // 2             # head pairs per core = 2
DT = D // 128               # 8 d-tiles
TT = S // 128               # 16 token tiles
QB = S // 512               # 4 q blocks
KT = S // 128               # 16 k tiles
SCALE = 0.125               # 1/sqrt(DEPTH)

SKEW = 16     # AV trails exp by this many flat steps (pe ring buffers)
VEND = 16     # flat steps reserved for interleaved V projection
NORM2D = 12   # steps from a unit's last AV to its bc broadcast
OPD = 20      # steps from a qb's last AV to its first out-proj chain
OPSPACE = 2


def build_nc():
    nc = bacc.Bacc(None, target_bir_lowering=False)

    xq = nc.dram_tensor("xq", [D, S], BF16, kind="ExternalInput")
    xk = nc.dram_tensor("xk", [D, S], BF16, kind="ExternalInput")
    xv = nc.dram_tensor("xv", [D, S], BF16, kind="ExternalInput")
    wq = nc.dram_tensor("wq", [D, C], BF16, kind="ExternalInput")
    wk = nc.dram_tensor("wk", [D, C], BF16, kind="ExternalInput")
    wv = nc.dram_tensor("wv", [D, C], BF16, kind="ExternalInput")
    wo = nc.dram_tensor("wo", [C, D], BF16, kind="ExternalInput")
    bq = nc.dram_tensor("bq", [C], F32, kind="ExternalInput")
    bk = nc.dram_tensor("bk", [C], F32, kind="ExternalInput")
    bv = nc.dram_tensor("bv", [C], F32, kind="ExternalInput")
    expb = nc.dram_tensor("expb", [S], F32, kind="ExternalInput")
    out = nc.dram_tensor("out", [S, D], F32, kind="ExternalOutput")

    with tile.TileContext(nc) as tc, nc.allow_low_precision(
        reason="bf16 storage with fp32 PSUM accumulation throughout"
    ):
        with (
            tc.tile_pool(name="wpool", bufs=1) as wp,
            tc.tile_pool(name="qk", bufs=1) as qkp,
            tc.tile_pool(name="vsb", bufs=1) as vp_,
            tc.tile_pool(name="ctxp", bufs=1) as cxp,
            tc.tile_pool(name="xs", bufs=16) as xsp,
            tc.tile_pool(name="vep", bufs=2) as vep,
            tc.tile_pool(name="pex", bufs=SKEW + 4) as pex,
            tc.tile_pool(name="nrm", bufs=2) as nrm,
            tc.tile_pool(name="osb", bufs=3) as osb,
        ):
            # ---- DMA issue order on the sync FIFO sets stream pacing ----
            wk_sb = wp.tile([128, DT, C], BF16)
            wq_sb = wp.tile([128, DT, C], BF16)
            nc.sync.dma_start(wk_sb[:], wk.rearrange("(dt p) c -> p dt c", p=128))
            nc.sync.dma_start(wq_sb[:], wq.rearrange("(dt p) c -> p dt c", p=128))
            xkr = xk.rearrange("(dt p) t -> dt p t", p=128)
            xqr = xq.rearrange("(dt p) t -> dt p t", p=128)
            xvr = xv.rearrange("(dt p) t -> dt p t", p=128)
            xk_t, xq_t, xv_t = [], [], []
            for dt in range(DT):
                xt = xsp.tile([128, S], BF16, tag="xt", name="xkt")
                nc.sync.dma_start(xt[:], xkr[dt])
                xk_t.append(xt)
            bq_sb = wp.tile([128, 2], F32)
            bk_sb = wp.tile([128, 2], F32)
            nc.sync.dma_start(bq_sb[:], bq.rearrange("(ct p) -> p ct", p=128))
            nc.sync.dma_start(bk_sb[:], bk.rearrange("(ct p) -> p ct", p=128))
            for dt in range(DT):
                xt = xsp.tile([128, S], BF16, tag="xt", name="xqt")
                nc.sync.dma_start(xt[:], xqr[dt])
                xq_t.append(xt)
            bvr_sb = wp.tile([128, C], F32)
            bv_ap = bv[:]
            bv_bc = bass.AP(tensor=bv_ap.tensor, offset=bv_ap.offset,
                            ap=[[0, 128]] + [list(a) for a in bv_ap.ap])
            nc.sync.dma_start(bvr_sb[:], bv_bc)
            expb_sb = wp.tile([128, TT], F32)
            nc.sync.dma_start(expb_sb[:], expb.rearrange("(tt p) -> p tt", p=128))
            wv_sb = wp.tile([128, DT, C], BF16)
            nc.sync.dma_start(wv_sb[:], wv.rearrange("(dt p) c -> p dt c", p=128))
            for dt in range(DT):
                xt = xsp.tile([128, S], BF16, tag="xt", name="xvt")
                nc.sync.dma_start(xt[:], xvr[dt])
                xv_t.append(xt)
            wo_sb = wp.tile([128, 2, D], BF16)
            nc.sync.dma_start(wo_sb[:], wo.rearrange("(ct p) n -> p ct n", p=128))

            ones65 = wp.tile([65, 64], BF16)
            nc.gpsimd.memset(ones65[:], 1.0)

            # ---- PE warm-up: trips HAM to K=8/8 before the projections ----
            with tc.tile_pool(name="warm", bufs=1, space="PSUM") as wmp:
                wps_t = wmp.tile([64, 64], F32)
                for _ in range(44):
                    nc.tensor.matmul(
                        wps_t[:], ones65[64:65, 0:64], ones65[64:65, 0:64],
                        start=True, stop=True,
                    )

            # ---- persistent activations ----
            qT = [qkp.tile([128, S], BF16, tag=f"qT{i}", name=f"qT{i}") for i in range(NPAIR)]
            kT = [qkp.tile([128, S], BF16, tag=f"kT{i}", name=f"kT{i}") for i in range(NPAIR)]
            vs = [vp_.tile([128, HC, 65], BF16, tag=f"vs{t}", name=f"vs{t}") for t in range(TT)]
            ctx = [cxp.tile([128, S], BF16, tag=f"ctx{i}", name=f"ctx{i}") for i in range(NPAIR)]

            # ================= K / Q projections (chase their streams) ======
            with tc.tile_pool(name="pp", bufs=8, space="PSUM") as pp:
                for xts, w_sb, b_sb, dst in (
                    (xk_t, wk_sb, bk_sb, kT),
                    (xq_t, wq_sb, bq_sb, qT),
                ):
                    ps = {}
                    for dt in range(DT):
                        xt = xts[dt]
                        for ct in range(2):
                            for tb in range(QB):
                                if dt == 0:
                                    ps[ct, tb] = pp.tile([128, 512], F32, tag="pp", name=f"ps{ct}_{tb}")
                                nc.tensor.matmul(
                                    ps[ct, tb][:],
                                    w_sb[:, dt, ct * 128:(ct + 1) * 128],
                                    xt[:, tb * 512:(tb + 1) * 512],
                                    start=(dt == 0),
                                    stop=(dt == DT - 1),
                                )
                                if dt == DT - 1:
                                    nc.vector.tensor_scalar_add(
                                        dst[ct][:, tb * 512:(tb + 1) * 512],
                                        ps[ct, tb][:],
                                        b_sb[:, ct:ct + 1],
                                    )

            # ========== attention w/ interleaved V-proj + output proj =======
            oout = out.rearrange("(qt p) n -> qt p n", p=128)
            flat = [(qb, pr, kt) for qb in range(QB) for pr in range(NPAIR)
                    for kt in range(KT)]
            NF = len(flat)
            pending = {}
            avs = {}
            norm_state = {}

            # ---- V-projection work schedule over the first VEND steps ----
            # quarter 0 (toks 0-3) chases the xv stream dt by dt;
            # quarters 1-3 (toks 4-15) run from resident tiles, one tok/step.
            vsched = {}
            for dt in range(DT):
                vsched.setdefault(min(dt, VEND - 1), []).append(("q0dt", dt))
            for t in range(4, TT):
                vsched.setdefault(min(8 + (t - 4) * 8 // 12, VEND - 1),
                                  []).append(("tok", t))

            norm2_sched = {}
            op_sched = {}
            for u in range(QB * NPAIR):
                pos = (u + 1) * KT - 1 + SKEW + NORM2D
                norm2_sched.setdefault(pos, []).append((u // NPAIR, u % NPAIR))
            for qb in range(QB):
                last_av = (qb + 1) * NPAIR * KT - 1 + SKEW
                for j in range(8):
                    op_sched.setdefault(last_av + OPD + j * OPSPACE, []).append((qb, j))

            with tc.tile_pool(name="sps", bufs=2, space="PSUM") as sps:

                def emit_qk(qb, pr, kt):
                    qsl = slice(qb * 512, (qb + 1) * 512)
                    ksl = slice(kt * 128, (kt + 1) * 128)
                    st = sps.tile([128, 1024], F32, tag="s", name="st")
                    for hh in range(2):
                        psl = slice(hh * 64, (hh + 1) * 64)
                        nc.tensor.matmul(
                            st[:, hh * 512:(hh + 1) * 512],
                            kT[pr][psl, ksl],
                            qT[pr][psl, qsl],
                        )
                    pe = pex.tile([128, 1024], BF16, tag="pe", name="pe")
                    nc.scalar.activation(
                        pe[:], st[:], mybir.ActivationFunctionType.Exp, scale=SCALE
                    )
                    pending[qb, pr, kt] = pe

                def emit_norm(qb, pr):
                    av = avs.pop((qb, pr))
                    avc = []
                    recs = []
                    for hh in range(2):
                        a = nrm.tile([65, 512], F32, tag="avc", bufs=4, name="avc")
                        nc.vector.tensor_copy(a[:], av[hh][:])
                        avc.append(a)
                    for hh in range(2):
                        rec = nrm.tile([65, 512], BF16, tag="rec", bufs=4, name="rec")
                        nc.vector.reciprocal(rec[64:65, :], avc[hh][64:65, :])
                        recs.append(rec)
                    norm_state[qb, pr] = (avc, recs)

                def emit_vtok_epi(vpt, tt):
                    tmpv = vep.tile([128, C], F32, tag="tmpv", name="tmpv")
                    nc.vector.tensor_add(tmpv[:], vpt[:], bvr_sb[:])
                    nc.vector.tensor_scalar_mul(
                        vs[tt][:, :, 0:64],
                        tmpv[:].rearrange("p (h d) -> p h d", h=HC),
                        expb_sb[:, tt:tt + 1],
                    )
                    for h in range(HC):
                        nc.gpsimd.tensor_copy(
                            vs[tt][:, h, 64:65], expb_sb[:, tt:tt + 1]
                        )

                # ---- phase A: QK/exp + interleaved V projection ----
                with tc.tile_pool(name="vps", bufs=4, space="PSUM") as vpp:
                    vq0 = [vpp.tile([128, C], F32, tag="vps", name=f"vq{j}")
                           for j in range(4)]
                    for i in range(VEND):
                        emit_qk(*flat[i])
                        for kind, a in vsched.get(i, ()):
                            if kind == "q0dt":
                                dt = a
                                for tok in range(4):
                                    nc.tensor.matmul(
                                        vq0[tok][:],
                                        xv_t[dt][:, tok * 128:(tok + 1) * 128],
                                        wv_sb[:, dt, :],
                                        start=(dt == 0),
                                        stop=(dt == DT - 1),
                                    )
                                if dt == DT - 1:
                                    for tok in range(4):
                                        emit_vtok_epi(vq0[tok], tok)
                            else:
                                t = a
                                vpt = vpp.tile([128, C], F32, tag="vps", name="vpt")
                                for dt in range(DT):
                                    nc.tensor.matmul(
                                        vpt[:],
                                        xv_t[dt][:, t * 128:(t + 1) * 128],
                                        wv_sb[:, dt, :],
                                        start=(dt == 0),
                                        stop=(dt == DT - 1),
                                    )
                                emit_vtok_epi(vpt, t)

                # ---- phase B: steady state + drain ----
                with tc.tile_pool(name="wps", bufs=4, space="PSUM") as wps:

                    def emit_av(qb, pr, kt):
                        pe = pending.pop((qb, pr, kt))
                        if kt == 0:
                            avs[qb, pr] = [
                                wps.tile([65, 512], F32, tag="w", name="av")
                                for _ in range(2)
                            ]
                        av = avs[qb, pr]
                        for hh in range(2):
                            nc.tensor.matmul(
                                av[hh][:],
                                vs[kt][:, pr * 2 + hh, :],
                                pe[:, hh * 512:(hh + 1) * 512],
                                start=(kt == 0),
                                stop=(kt == KT - 1),
                            )
                        if kt == KT - 1:
                            emit_norm(qb, pr)

                    def emit_norm2(qb, pr):
                        qsl = slice(qb * 512, (qb + 1) * 512)
                        avc, recs = norm_state.pop((qb, pr))
                        for hh in range(2):
                            bc = wps.tile([128, 512], F32, tag="w", name="bc")
                            nc.tensor.matmul(
                                bc[0:64, :], ones65[64:65, 0:64],
                                recs[hh][64:65, :],
                                start=True, stop=True,
                            )
                            if hh == 0:
                                nc.vector.tensor_mul(
                                    ctx[pr][0:64, qsl], avc[hh][0:64, :],
                                    bc[0:64, :]
                                )
                            else:
                                tmp = nrm.tile([64, 512], BF16, tag="tmp",
                                               name="tmp")
                                nc.vector.tensor_mul(tmp[:], avc[hh][0:64, :],
                                                     bc[0:64, :])
                                nc.sync.dma_start(ctx[pr][64:128, qsl], tmp[:])

                    def emit_outproj_chain(qb, j):
                        qt = qb * 4 + j // 2
                        n = j % 2
                        qts = slice(qt * 128, (qt + 1) * 128)
                        po = wps.tile([128, 512], F32, tag="w", name="po")
                        for ct in range(2):
                            nc.tensor.matmul(
                                po[:],
                                ctx[ct][:, qts],
                                wo_sb[:, ct, n * 512:(n + 1) * 512],
                                start=(ct == 0),
                                stop=(ct == 1),
                            )
                        ot = osb.tile([128, 512], F32, tag="ot", name="ot")
                        nc.vector.tensor_copy(ot[:], po[:])
                        nc.sync.dma_start(oout[qt][:, n * 512:(n + 1) * 512],
                                          ot[:])

                    emitted_ops = set()
                    for i in range(VEND, NF + SKEW):
                        if i < NF:
                            emit_qk(*flat[i])
                        for key in norm2_sched.get(i, ()):
                            emit_norm2(*key)
                        if i >= SKEW:
                            emit_av(*flat[i - SKEW])
                        for key in op_sched.get(i, ()):
                            emit_outproj_chain(*key)
                            emitted_ops.add(key)
                    for key in sorted(norm_state):
                        emit_norm2(*key)
                    for qb in range(QB):
                        for j in range(8):
                            if (qb, j) not in emitted_ops:
                                emit_outproj_chain(qb, j)

    nc.finalize()
    return nc


_NC = None


def _get_nc():
    global _NC
    if _NC is None:
        _NC = build_nc()
    return _NC


def make_in_maps(query, key, value, temporal_bias, wq, bq, wk, bk, wv, bv, wo, bo):
    f = np.float32
    bf = ml_dtypes.bfloat16
    xt = {}
    for b in range(B):
        xt["q", b] = np.ascontiguousarray(np.asarray(query[b], f).T.astype(bf))
        xt["k", b] = np.ascontiguousarray(np.asarray(key[b], f).T.astype(bf))
        xt["v", b] = np.ascontiguousarray(np.asarray(value[b], f).T.astype(bf))
    expb = np.exp(np.asarray(temporal_bias, f))
    wq = np.asarray(wq, f).astype(bf)
    wk = np.asarray(wk, f).astype(bf)
    wv = np.asarray(wv, f).astype(bf)
    wo = np.asarray(wo, f).astype(bf)
    in_maps = []
    for core in range(NCORES):
        b, g = divmod(core, GROUPS)
        cs = slice(g * C, (g + 1) * C)
        in_maps.append({
            "xq": xt["q", b],
            "xk": xt["k", b],
            "xv": xt["v", b],
            "wq": np.ascontiguousarray(wq[:, cs]),
            "wk": np.ascontiguousarray(wk[:, cs]),
            "wv": np.ascontiguousarray(wv[:, cs]),
            "wo": np.ascontiguousarray(wo[cs, :]),
            "bq": np.ascontiguousarray(np.asarray(bq, f)[cs]),
            "bk": np.ascontiguousarray(np.asarray(bk, f)[cs]),
            "bv": np.ascontiguousarray(np.asarray(bv, f)[cs]),
            "expb": np.ascontiguousarray(expb[b]),
        })
    return in_maps


def gather(results, bo):
    bo = np.asarray(bo, np.float32)
    out = np.zeros((B, S, D), np.float32)
    for core in range(NCORES):
        b = core // GROUPS
        out[b] += results[core]["out"]
    out += bo[None, None, :]
    return out


def kernel(query, key, value, temporal_bias, wq, bq, wk, bk, wv, bv, wo, bo,
           _trace=False):
    nc = _get_nc()
    in_maps = make_in_maps(query, key, value, temporal_bias,
                           wq, bq, wk, bk, wv, bv, wo, bo)
    res = run_bass_kernel_spmd(nc, in_maps, list(range(NCORES)), trace=_trace)
    out = gather(res.results, bo)
    if _trace:
        return out, res
    return out


# revision 4
# speedup vs baseline: 1.0088x; 1.0061x over previous
"""EnhancedMultiHeadAttention on 8 Trainium2 NeuronCores — v2.

Sharding: 8 cores = 2 batches x 4 head-groups (4 heads / 256 columns each).

Design notes:
- bf16 on-chip everywhere (host-cast bf16 inputs); fp32 PSUM accumulation.
- Stream order xk, xq, xv: K/Q projections chase their streams so the
  exp pipeline (the throughput bottleneck: 128 x ~1.15us ScalarE
  activations) starts as soon as xq lands.  The V projection is
  interleaved into the first ~20 attention steps while xv streams, and
  AV matmuls trail the exp stream by SKEW=20 steps (pe tiles buffer in
  a deep SBUF ring).
- All x tiles go through one 16-slot SBUF ring; xv reuses xk's slots.
- V projection computed directly in [token, channel] layout (lhsT = x
  tile) in 4-bank PSUM quarters; no PE transposes.
- Projection bias epilogues on VectorE so ScalarE does exp only.
- Softmax normalization: copy av PSUM->SBUF (frees the PSUM ring fast),
  reciprocal of the denominator row, then a K=1 PE matmul broadcasts
  the reciprocal row across 64 partitions — no DRAM bounce.  The PE
  broadcast is scheduled 16 steps later so it never waits on the
  reciprocal.
- exp(temporal_bias) folded into V' and the denominator column.
- A PE warm-up burst trips the HAM clock gate to 8/8 before the
  projections start.
"""

import sys

for _p in ("/opt/trn_rl_repo", "/root/.axon_site/_ro/trn_rl_repo"):
    if _p not in sys.path:
        sys.path.append(_p)

import numpy as np
import ml_dtypes

import concourse.bass as bass
import concourse.mybir as mybir
import concourse.tile as tile
from concourse import bacc
from concourse.bass_utils import run_bass_kernel_spmd

F32 = mybir.dt.float32
BF16 = mybir.dt.bfloat16

B, S, D = 2, 2048, 1024
H, DEPTH = 16, 64
NCORES = 8
GROUPS = 4                  # head-groups per batch
HC = H // GROUPS            # heads per core = 4
C = HC * DEPTH              # columns per core = 256
NPAIR = HC // 2             # head pairs per core = 2
DT = D // 128               # 8 d-tiles
TT = S // 128               # 16 token tiles
QB = S // 512               # 4 q blocks
KT = S // 128               # 16 k tiles
SCALE = 0.125               # 1/sqrt(DEPTH)

SKEW = 16     # AV trails exp by this many flat steps (pe ring buffers)
VEND = 16     # flat steps reserved for interleaved V projection
NORM2D = 12   # steps from a unit's last AV to its bc broadcast
OPD = 20      # steps from a qb's last AV to its first out-proj chain
OPSPACE = 2


def build_nc():
    nc = bacc.Bacc(None, target_bir_lowering=False)

    xq = nc.dram_tensor("xq", [D, S], BF16, kind="ExternalInput")
    xk = nc.dram_tensor("xk", [D, S], BF16, kind="ExternalInput")
    xv = nc.dram_tensor("xv", [D, S], BF16, kind="ExternalInput")
    wq = nc.dram_tensor("wq", [D, C], BF16, kind="ExternalInput")
    wk = nc.dram_tensor("wk", [D, C], BF16, kind="ExternalInput")
    wv = nc.dram_tensor("wv", [D, C], BF16, kind="ExternalInput")
    wo = nc.dram_tensor("wo", [C, D], BF16, kind="ExternalInput")
    bq = nc.dram_tensor("bq", [C], F32, kind="ExternalInput")
    bk = nc.dram_tensor("bk", [C], F32, kind="ExternalInput")
    bv = nc.dram_tensor("bv", [C], F32, kind="ExternalInput")
    expb = nc.dram_tensor("expb", [S], F32, kind="ExternalInput")
    out = nc.dram_tensor("out", [S, D], F32, kind="ExternalOutput")

    with tile.TileContext(nc) as tc, nc.allow_low_precision(
        reason="bf16 storage with fp32 PSUM accumulation throughout"
    ):
        with (
            tc.tile_pool(name="wpool", bufs=1) as wp,
            tc.tile_pool(name="qk", bufs=1) as qkp,
            tc.tile_pool(name="vsb", bufs=1) as vp_,
            tc.tile_pool(name="ctxp", bufs=1) as cxp,
            tc.tile_pool(name="xs", bufs=16) as xsp,
            tc.tile_pool(name="vep", bufs=2) as vep,
            tc.tile_pool(name="pex", bufs=SKEW + 4) as pex,
            tc.tile_pool(name="nrm", bufs=2) as nrm,
            tc.tile_pool(name="osb", bufs=3) as osb,
        ):
            # ---- DMA issue order on the sync FIFO sets stream pacing ----
            wk_sb = wp.tile([128, DT, C], BF16)
            wq_sb = wp.tile([128, DT, C], BF16)
            nc.sync.dma_start(wk_sb[:], wk.rearrange("(dt p) c -> p dt c", p=128))
            nc.sync.dma_start(wq_sb[:], wq.rearrange("(dt p) c -> p dt c", p=128))
            xkr = xk.rearrange("(dt p) t -> dt p t", p=128)
            xqr = xq.rearrange("(dt p) t -> dt p t", p=128)
            xvr = xv.rearrange("(dt p) t -> dt p t", p=128)
            xk_t, xq_t, xv_t = [], [], []
            for dt in range(DT):
                xt = xsp.tile([128, S], BF16, tag="xt", name="xkt")
                nc.sync.dma_start(xt[:], xkr[dt])
                xk_t.append(xt)
            bq_sb = wp.tile([128, 2], F32)
            bk_sb = wp.tile([128, 2], F32)
            nc.sync.dma_start(bq_sb[:], bq.rearrange("(ct p) -> p ct", p=128))
            nc.sync.dma_start(bk_sb[:], bk.rearrange("(ct p) -> p ct", p=128))
            for dt in range(DT):
                xt = xsp.tile([128, S], BF16, tag="xt", name="xqt")
                nc.sync.dma_start(xt[:], xqr[dt])
                xq_t.append(xt)
            bvr_sb = wp.tile([128, C], F32)
            bv_ap = bv[:]
            bv_bc = bass.AP(tensor=bv_ap.tensor, offset=bv_ap.offset,
                            ap=[[0, 128]] + [list(a) for a in bv_ap.ap])
            nc.sync.dma_start(bvr_sb[:], bv_bc)
            expb_sb = wp.tile([128, TT], F32)
            nc.sync.dma_start(expb_sb[:], expb.rearrange("(tt p) -> p tt", p=128))
            wv_sb = wp.tile([128, DT, C], BF16)
            nc.sync.dma_start(wv_sb[:], wv.rearrange("(dt p) c -> p dt c", p=128))
            for dt in range(DT):
                xt = xsp.tile([128, S], BF16, tag="xt", name="xvt")
                nc.sync.dma_start(xt[:], xvr[dt])
                xv_t.append(xt)
            wo_sb = wp.tile([128, 2, D], BF16)
            nc.sync.dma_start(wo_sb[:], wo.rearrange("(ct p) n -> p ct n", p=128))

            ones65 = wp.tile([65, 64], BF16)
            nc.gpsimd.memset(ones65[:], 1.0)

            # ---- PE warm-up: trips HAM to K=8/8 before the projections ----
            with tc.tile_pool(name="warm", bufs=1, space="PSUM") as wmp:
                wps_t = wmp.tile([64, 64], F32)
                for _ in range(44):
                    nc.tensor.matmul(
                        wps_t[:], ones65[64:65, 0:64], ones65[64:65, 0:64],
                        start=True, stop=True,
                    )

            # ---- persistent activations ----
            qT = [qkp.tile([128, S], BF16, tag=f"qT{i}", name=f"qT{i}") for i in range(NPAIR)]
            kT = [qkp.tile([128, S], BF16, tag=f"kT{i}", name=f"kT{i}") for i in range(NPAIR)]
            vs = [vp_.tile([128, HC, 65], BF16, tag=f"vs{t}", name=f"vs{t}") for t in range(TT)]
            ctx = [cxp.tile([128, S], BF16, tag=f"ctx{i}", name=f"ctx{i}") for i in range(NPAIR)]

            # ================= K / Q projections (chase their streams) ======
            with tc.tile_pool(name="pp", bufs=8, space="PSUM") as pp:
                for xts, w_sb, b_sb, dst in (
                    (xk_t, wk_sb, bk_sb, kT),
                    (xq_t, wq_sb, bq_sb, qT),
                ):
                    ps = {}
                    for dt in range(DT):
                        xt = xts[dt]
                        for ct in range(2):
                            for tb in range(QB):
                                if dt == 0:
                                    ps[ct, tb] = pp.tile([128, 512], F32, tag="pp", name=f"ps{ct}_{tb}")
                                nc.tensor.matmul(
                                    ps[ct, tb][:],
                                    w_sb[:, dt, ct * 128:(ct + 1) * 128],
                                    xt[:, tb * 512:(tb + 1) * 512],
                                    start=(dt == 0),
                                    stop=(dt == DT - 1),
                                )
                                if dt == DT - 1:
                                    nc.vector.tensor_scalar_add(
                                        dst[ct][:, tb * 512:(tb + 1) * 512],
                                        ps[ct, tb][:],
                                        b_sb[:, ct:ct + 1],
                                    )

            # ========== attention w/ interleaved V-proj + output proj =======
            oout = out.rearrange("(qt p) n -> qt p n", p=128)
            flat = [(qb, pr, kt) for qb in range(QB) for pr in range(NPAIR)
                    for kt in range(KT)]
            NF = len(flat)
            pending = {}
            avs = {}
            norm_state = {}

            # ---- V-projection work schedule over the first VEND steps ----
            # quarter 0 (toks 0-3) chases the xv stream dt by dt;
            # quarters 1-3 (toks 4-15) run from resident tiles, one tok/step.
            vsched = {}
            for dt in range(DT):
                vsched.setdefault(min(dt, VEND - 1), []).append(("q0dt", dt))
            for t in range(4, TT):
                vsched.setdefault(min(8 + (t - 4) * 8 // 12, VEND - 1),
                                  []).append(("tok", t))

            norm2_sched = {}
            recip_sched = {}
            op_sched = {}
            for u in range(QB * NPAIR):
                pos = (u + 1) * KT - 1 + SKEW + NORM2D
                norm2_sched.setdefault(pos, []).append((u // NPAIR, u % NPAIR))
                rpos = (u + 1) * KT - 1 + SKEW + 4
                recip_sched.setdefault(rpos, []).append((u // NPAIR, u % NPAIR))
            for qb in range(QB):
                last_av = (qb + 1) * NPAIR * KT - 1 + SKEW
                for j in range(8):
                    op_sched.setdefault(last_av + OPD + j * OPSPACE, []).append((qb, j))

            with tc.tile_pool(name="sps", bufs=2, space="PSUM") as sps:

                def emit_qk(qb, pr, kt):
                    qsl = slice(qb * 512, (qb + 1) * 512)
                    ksl = slice(kt * 128, (kt + 1) * 128)
                    st = sps.tile([128, 1024], F32, tag="s", name="st")
                    for hh in range(2):
                        psl = slice(hh * 64, (hh + 1) * 64)
                        nc.tensor.matmul(
                            st[:, hh * 512:(hh + 1) * 512],
                            kT[pr][psl, ksl],
                            qT[pr][psl, qsl],
                        )
                    pe = pex.tile([128, 1024], BF16, tag="pe", name="pe")
                    nc.scalar.activation(
                        pe[:], st[:], mybir.ActivationFunctionType.Exp, scale=SCALE
                    )
                    pending[qb, pr, kt] = pe

                def emit_norm(qb, pr):
                    av = avs.pop((qb, pr))
                    avc = []
                    for hh in range(2):
                        a = nrm.tile([65, 512], F32, tag="avc", bufs=4, name="avc")
                        nc.vector.tensor_copy(a[:], av[hh][:])
                        avc.append(a)
                    norm_state[qb, pr] = (avc, [])

                def emit_recips(qb, pr):
                    avc, recs = norm_state[qb, pr]
                    for hh in range(2):
                        rec = nrm.tile([65, 512], BF16, tag="rec", bufs=4, name="rec")
                        nc.vector.reciprocal(rec[64:65, :], avc[hh][64:65, :])
                        recs.append(rec)

                def emit_vtok_epi(vpt, tt):
                    tmpv = vep.tile([128, C], F32, tag="tmpv", name="tmpv")
                    nc.vector.tensor_add(tmpv[:], vpt[:], bvr_sb[:])
                    nc.vector.tensor_scalar_mul(
                        vs[tt][:, :, 0:64],
                        tmpv[:].rearrange("p (h d) -> p h d", h=HC),
                        expb_sb[:, tt:tt + 1],
                    )
                    for h in range(HC):
                        nc.gpsimd.tensor_copy(
                            vs[tt][:, h, 64:65], expb_sb[:, tt:tt + 1]
                        )

                # ---- phase A: QK/exp + interleaved V projection ----
                with tc.tile_pool(name="vps", bufs=4, space="PSUM") as vpp:
                    vq0 = [vpp.tile([128, C], F32, tag="vps", name=f"vq{j}")
                           for j in range(4)]
                    for i in range(VEND):
                        emit_qk(*flat[i])
                        for kind, a in vsched.get(i, ()):
                            if kind == "q0dt":
                                dt = a
                                for tok in range(4):
                                    nc.tensor.matmul(
                                        vq0[tok][:],
                                        xv_t[dt][:, tok * 128:(tok + 1) * 128],
                                        wv_sb[:, dt, :],
                                        start=(dt == 0),
                                        stop=(dt == DT - 1),
                                    )
                                if dt == DT - 1:
                                    for tok in range(4):
                                        emit_vtok_epi(vq0[tok], tok)
                            else:
                                t = a
                                vpt = vpp.tile([128, C], F32, tag="vps", name="vpt")
                                for dt in range(DT):
                                    nc.tensor.matmul(
                                        vpt[:],
                                        xv_t[dt][:, t * 128:(t + 1) * 128],
                                        wv_sb[:, dt, :],
                                        start=(dt == 0),
                                        stop=(dt == DT - 1),
                                    )
                                emit_vtok_epi(vpt, t)

                # ---- phase B: steady state + drain ----
                with tc.tile_pool(name="wps", bufs=4, space="PSUM") as wps:

                    def emit_av(qb, pr, kt):
                        pe = pending.pop((qb, pr, kt))
                        if kt == 0:
                            avs[qb, pr] = [
                                wps.tile([65, 512], F32, tag="w", name="av")
                                for _ in range(2)
                            ]
                        av = avs[qb, pr]
                        for hh in range(2):
                            nc.tensor.matmul(
                                av[hh][:],
                                vs[kt][:, pr * 2 + hh, :],
                                pe[:, hh * 512:(hh + 1) * 512],
                                start=(kt == 0),
                                stop=(kt == KT - 1),
                            )
                        if kt == KT - 1:
                            emit_norm(qb, pr)

                    def emit_norm2(qb, pr):
                        qsl = slice(qb * 512, (qb + 1) * 512)
                        avc, recs = norm_state.pop((qb, pr))
                        for hh in range(2):
                            bc = wps.tile([128, 512], F32, tag="w", name="bc")
                            nc.tensor.matmul(
                                bc[0:64, :], ones65[64:65, 0:64],
                                recs[hh][64:65, :],
                                start=True, stop=True,
                            )
                            if hh == 0:
                                nc.vector.tensor_mul(
                                    ctx[pr][0:64, qsl], avc[hh][0:64, :],
                                    bc[0:64, :]
                                )
                            else:
                                tmp = nrm.tile([64, 512], BF16, tag="tmp",
                                               name="tmp")
                                nc.vector.tensor_mul(tmp[:], avc[hh][0:64, :],
                                                     bc[0:64, :])
                                nc.sync.dma_start(ctx[pr][64:128, qsl], tmp[:])

                    def emit_outproj_chain(qb, j):
                        qt = qb * 4 + j // 2
                        n = j % 2
                        qts = slice(qt * 128, (qt + 1) * 128)
                        po = wps.tile([128, 512], F32, tag="w", name="po")
                        for ct in range(2):
                            nc.tensor.matmul(
                                po[:],
                                ctx[ct][:, qts],
                                wo_sb[:, ct, n * 512:(n + 1) * 512],
                                start=(ct == 0),
                                stop=(ct == 1),
                            )
                        ot = osb.tile([128, 512], F32, tag="ot", name="ot")
                        nc.vector.tensor_copy(ot[:], po[:])
                        nc.sync.dma_start(oout[qt][:, n * 512:(n + 1) * 512],
                                          ot[:])

                    emitted_ops = set()
                    for i in range(VEND, NF + SKEW):
                        if i < NF:
                            emit_qk(*flat[i])
                        for key in recip_sched.get(i, ()):
                            emit_recips(*key)
                        for key in norm2_sched.get(i, ()):
                            emit_norm2(*key)
                        if i >= SKEW:
                            emit_av(*flat[i - SKEW])
                        for key in op_sched.get(i, ()):
                            emit_outproj_chain(*key)
                            emitted_ops.add(key)
                    for pos in sorted(recip_sched):
                        if pos >= NF + SKEW:
                            for key in recip_sched[pos]:
                                emit_recips(*key)
                    for key in sorted(norm_state):
                        emit_norm2(*key)
                    for qb in range(QB):
                        for j in range(8):
                            if (qb, j) not in emitted_ops:
                                emit_outproj_chain(qb, j)

    nc.finalize()
    return nc


_NC = None


def _get_nc():
    global _NC
    if _NC is None:
        _NC = build_nc()
    return _NC


def make_in_maps(query, key, value, temporal_bias, wq, bq, wk, bk, wv, bv, wo, bo):
    f = np.float32
    bf = ml_dtypes.bfloat16
    xt = {}
    for b in range(B):
        xt["q", b] = np.ascontiguousarray(np.asarray(query[b], f).T.astype(bf))
        xt["k", b] = np.ascontiguousarray(np.asarray(key[b], f).T.astype(bf))
        xt["v", b] = np.ascontiguousarray(np.asarray(value[b], f).T.astype(bf))
    expb = np.exp(np.asarray(temporal_bias, f))
    wq = np.asarray(wq, f).astype(bf)
    wk = np.asarray(wk, f).astype(bf)
    wv = np.asarray(wv, f).astype(bf)
    wo = np.asarray(wo, f).astype(bf)
    in_maps = []
    for core in range(NCORES):
        b, g = divmod(core, GROUPS)
        cs = slice(g * C, (g + 1) * C)
        in_maps.append({
            "xq": xt["q", b],
            "xk": xt["k", b],
            "xv": xt["v", b],
            "wq": np.ascontiguousarray(wq[:, cs]),
            "wk": np.ascontiguousarray(wk[:, cs]),
            "wv": np.ascontiguousarray(wv[:, cs]),
            "wo": np.ascontiguousarray(wo[cs, :]),
            "bq": np.ascontiguousarray(np.asarray(bq, f)[cs]),
            "bk": np.ascontiguousarray(np.asarray(bk, f)[cs]),
            "bv": np.ascontiguousarray(np.asarray(bv, f)[cs]),
            "expb": np.ascontiguousarray(expb[b]),
        })
    return in_maps


def gather(results, bo):
    bo = np.asarray(bo, np.float32)
    out = np.zeros((B, S, D), np.float32)
    for core in range(NCORES):
        b = core // GROUPS
        out[b] += results[core]["out"]
    out += bo[None, None, :]
    return out


def kernel(query, key, value, temporal_bias, wq, bq, wk, bk, wv, bv, wo, bo,
           _trace=False):
    nc = _get_nc()
    in_maps = make_in_maps(query, key, value, temporal_bias,
                           wq, bq, wk, bk, wv, bv, wo, bo)
    res = run_bass_kernel_spmd(nc, in_maps, list(range(NCORES)), trace=_trace)
    out = gather(res.results, bo)
    if _trace:
        return out, res
    return out
